# revision 2
# baseline (speedup 1.0000x reference)
"""Trainium2 Bass kernel v2 for nn_EstimatorQNN (MLP -> pairwise fidelity graph -> adj @ out).

Contract: kernel(**inputs) takes FULL unsharded inputs (numpy, fp32) and
returns the FULL [8192, 64] fp32 output.  Batch sharded across 8 cores;
host-side all-gather + normalize + fp8 quantize between the two launches.

Phase 1 (per core): outT = MLP(x_local).T          [pure MLP, no norm chain]
Phase 2 (per core): fid block-row = nf_blk.T @ nl (fp8), threshold spread
                    across ACT/DVE/GPS, yT += ob_blk @ adj accumulated.
"""

import numpy as np
import ml_dtypes

import concourse.bass as bass
import concourse.tile as tile
from concourse import bacc, mybir
from concourse.bass_utils import run_bass_kernel_spmd
from concourse.bass_interp import get_hw_module

F32 = mybir.dt.float32
BF16 = mybir.dt.bfloat16
FP8 = mybir.dt.float8e4
AF = mybir.ActivationFunctionType
ALU = mybir.AluOpType

B, D_IN, H1, H2, D_OUT = 8192, 256, 512, 256, 64
NCORES = 8
LOCAL = B // NCORES          # 1024 rows per core
THRESHOLD = 0.9
SQRT_T = float(np.sqrt(np.float32(THRESHOLD)))
BIG = 3.0e4                  # diag-kill threshold (bf16-representable, > any |dot|)


# ---------------------------------------------------------------------------
# Phase 1: MLP only.  Inputs (per core):
#   x2  [2, 128, 1024] bf16 (x_local.T chunked on the 256-dim)
#   w1  [2, 128, 512]  bf16     w2 [4, 128, 256]     w3 [2, 128, 64]
#   b1  [128, 4] f32            b2 [128, 2]          b3 [64, 1]
# Output: outT [64, 1024] f32
# ---------------------------------------------------------------------------
def build_phase1(n_b=LOCAL, reps=1):
    nb = n_b // 512          # batch chunks of 512
    nc = bacc.Bacc("TRN2", target_bir_lowering=False, debug=False,
                   enable_asserts=False, num_devices=NCORES)
    x2 = nc.dram_tensor("x2", [2, 128, n_b], BF16, kind="ExternalInput")
    w1 = nc.dram_tensor("w1", [2, 128, 512], BF16, kind="ExternalInput")
    w2 = nc.dram_tensor("w2", [4, 128, 256], BF16, kind="ExternalInput")
    w3 = nc.dram_tensor("w3", [2, 128, 64], BF16, kind="ExternalInput")
    b1 = nc.dram_tensor("b1", [128, 4], F32, kind="ExternalInput")
    b2 = nc.dram_tensor("b2", [128, 2], F32, kind="ExternalInput")
    b3 = nc.dram_tensor("b3", [64, 1], F32, kind="ExternalInput")
    outT = nc.dram_tensor("outT", [64, n_b], F32, kind="ExternalOutput")

    with tile.TileContext(nc) as tc:
        with (
            tc.tile_pool(name="wpool", bufs=1) as wpool,
            tc.tile_pool(name="hpool", bufs=1) as hpool,
            tc.tile_pool(name="ps128", bufs=4, space="PSUM") as ps128,
            tc.tile_pool(name="ps64", bufs=2, space="PSUM") as ps64,
        ):
            for rep in range(reps):
                xa_sb = wpool.tile([128, 2, 512], BF16, tag="xa")
                xb_sb = wpool.tile([128, 2, n_b - 512], BF16, tag="xb")
                w1_sb = wpool.tile([128, 2, 512], BF16, tag="w1")
                w2_sb = wpool.tile([128, 4, 256], BF16, tag="w2")
                w3_sb = wpool.tile([128, 2, 64], BF16, tag="w3")
                b1_sb = wpool.tile([128, 4], F32, tag="b1")
                b2_sb = wpool.tile([128, 2], F32, tag="b2")
                b3_sb = wpool.tile([64, 1], F32, tag="b3")

                def xs(kc, sl):
                    if sl.stop <= 512:
                        return xa_sb[:, kc, sl]
                    return xb_sb[:, kc, sl.start - 512:sl.stop - 512]

                # critical-path inputs first; first chunk as few DMAs as possible
                nc.sync.dma_start(w1_sb[:, 0, :], w1[0])
                nc.gpsimd.dma_start(xa_sb[:, 0, :], x2[0, :, 0:512])
                nc.sync.dma_start(w1_sb[:, 1, :], w1[1])
                nc.gpsimd.dma_start(xa_sb[:, 1, :], x2[1, :, 0:512])
                nc.scalar.dma_start(b1_sb[:], b1[:, :])
                for kc in range(4):
                    nc.scalar.dma_start(w2_sb[:, kc, :], w2[kc])
                nc.scalar.dma_start(b2_sb[:], b2[:, :])
                for kc in range(2):
                    nc.scalar.dma_start(w3_sb[:, kc, :], w3[kc])
                nc.scalar.dma_start(b3_sb[:], b3[:, :])
                if n_b > 512:
                    nc.gpsimd.dma_start(xb_sb[:, 0, :], x2[0, :, 512:n_b])
                    nc.gpsimd.dma_start(xb_sb[:, 1, :], x2[1, :, 512:n_b])

                h1_sb = hpool.tile([128, 4, n_b], BF16, tag="h1")
                h2_sb = hpool.tile([128, 2, n_b], BF16, tag="h2")
                out_sb = hpool.tile([64, n_b], F32, tag="out")

                # chunk-major pipeline: h2(bb) overlaps h1(bb+1)
                for bb in range(nb):
                    sl = slice(bb * 512, (bb + 1) * 512)
                    # h1T = tanh(W1 @ xT + b1): [512, 512chunk]
                    for hb in range(4):
                        ps = ps128.tile([128, 512], F32, tag="mm")
                        for kc in range(2):
                            nc.tensor.matmul(
                                ps[:],
                                w1_sb[:, kc, hb * 128:(hb + 1) * 128],
                                xs(kc, sl),
                                start=(kc == 0), stop=(kc == 1))
                        nc.scalar.activation(
                            h1_sb[:, hb, sl], ps[:],
                            AF.Tanh, bias=b1_sb[:, hb:hb + 1], scale=1.0)
                    # h2T = tanh(W2 @ h1T + b2): [256, 512chunk]
                    for hb in range(2):
                        ps = ps128.tile([128, 512], F32, tag="mm")
                        for kc in range(4):
                            nc.tensor.matmul(
                                ps[:],
                                w2_sb[:, kc, hb * 128:(hb + 1) * 128],
                                h1_sb[:, kc, sl],
                                start=(kc == 0), stop=(kc == 3))
                        nc.scalar.activation(
                            h2_sb[:, hb, sl], ps[:],
                            AF.Tanh, bias=b2_sb[:, hb:hb + 1], scale=1.0)
                    # outT = W3 @ h2T + b3
                    ps = ps64.tile([64, 512], F32, tag="mmo")
                    for kc in range(2):
                        nc.tensor.matmul(
                            ps[:], w3_sb[:, kc, :], h2_sb[:, kc, sl],
                            start=(kc == 0), stop=(kc == 1))
                    nc.scalar.activation(
                        out_sb[:, sl], ps[:],
                        AF.Identity, bias=b3_sb[:, 0:1], scale=1.0)
                    nc.sync.dma_start(outT[:, sl], out_sb[:, sl])

    nc.compile()
    return nc


# ---------------------------------------------------------------------------
# Phase 2: gram + threshold + adjacency matmul.  Inputs (per core):
#   nfull [64, 8192] fp8  -- normedT_full rolled by -1024*core (k axis)
#   nloc  [64, 1024] fp8  -- this core's normedT slice
#   obf   [128, 64, 64] bf16 -- out_full rolled likewise; [p, kb, d]
#   thr   [128, 2048] bf16 -- 2 diag-kill threshold variants
# Output: yt [64, 1024] f32  (yT for the local rows)
#
# Threshold engine assignment per pair step (npair=32 per mb):
#   diag pairs (2): DVE tensor_tensor is_ge vs thr (one pass from psum)
#   act_set:        ACT Abs -> DVE or GPS bf16 is_ge
#   dve_set:        DVE tensor_scalar is_ge from psum (one-sided)
# ---------------------------------------------------------------------------
SIG_K = 80.0                 # sigmoid sharpness: tail < 1e-8 at margin 0.23


def build_phase2(n_kb=B // 128, n_mb=LOCAL // 512, lag=10, reps=1,
                 n_dve_full=13):
    npair = n_kb // 2
    nc = bacc.Bacc("TRN2", target_bir_lowering=False, debug=False,
                   enable_asserts=False, num_devices=NCORES)
    nfull = nc.dram_tensor("nfull", [64, n_kb * 128], FP8, kind="ExternalInput")
    nloc = nc.dram_tensor("nloc", [64, n_mb * 512], FP8, kind="ExternalInput")
    obf = nc.dram_tensor("obf", [128, n_kb, 64], BF16, kind="ExternalInput")
    thr = nc.dram_tensor("thr", [128, 2048], BF16, kind="ExternalInput")
    yt = nc.dram_tensor("yt", [64, n_mb * 512], F32, kind="ExternalOutput")

    with tile.TileContext(nc) as tc:
        with (
            tc.tile_pool(name="big", bufs=1) as big,
            tc.tile_pool(name="adjp", bufs=lag + 4) as adjp,
            tc.tile_pool(name="absp", bufs=6) as absp,
            tc.tile_pool(name="outp", bufs=2) as outp,
            tc.tile_pool(name="fidp", bufs=3, space="PSUM") as fidp,
            tc.tile_pool(name="ytp", bufs=2, space="PSUM") as ytp,
        ):
            for rep in range(reps):
                # head tile = cols 512:2560 (pairs 2..9 = first 16 steps of
                # the reordered schedule) so the pipeline starts after 1 DMA
                # per half; diag cols 0:512 + the rest live in body tiles.
                HA, HB = 512, 2560
                nfa_sb = big.tile([128, HB - HA], FP8, tag="nfa")
                nfb0_sb = big.tile([128, HA], FP8, tag="nfb0")
                nfb1_sb = big.tile([128, n_kb * 128 - HB], FP8, tag="nfb1")
                nl_sb = big.tile([128, n_mb * 512], FP8, tag="nl")
                ob_sb = big.tile([128, n_kb, 64], BF16, tag="ob")
                th_sb = big.tile([128, 2048], BF16, tag="th")
                sbias = big.tile([128, 1], F32, tag="sbias")
                nc.vector.memset(sbias[:], -SIG_K * SQRT_T)

                def nf(lo, cols):
                    """nfull slice [lo partition half, col range] across tiles."""
                    if HA <= cols.start < HB:
                        return nfa_sb[lo:lo + 64,
                                      cols.start - HA:cols.stop - HA]
                    if cols.start < HA:
                        return nfb0_sb[lo:lo + 64, cols]
                    return nfb1_sb[lo:lo + 64,
                                   cols.start - HB:cols.stop - HB]

                # critical-path first: locals + head tile, few DMAs, then body
                nc.sync.dma_start(nl_sb[0:64, :], nloc[:, :])
                nc.gpsimd.dma_start(nl_sb[64:128, :], nloc[:, :])
                nc.sync.dma_start(nfa_sb[0:64, :], nfull[:, HA:HB])
                nc.gpsimd.dma_start(nfa_sb[64:128, :], nfull[:, HA:HB])
                nc.sync.dma_start(nfb0_sb[0:64, :], nfull[:, 0:HA])
                nc.gpsimd.dma_start(nfb0_sb[64:128, :], nfull[:, 0:HA])
                total = n_kb * 128
                edges = list(range(HB, total, 2048)) + [total]
                for i in range(len(edges) - 1):
                    ch, w = edges[i], edges[i + 1] - edges[i]
                    dst = slice(ch - HB, ch + w - HB)
                    ea = nc.sync if i % 2 == 0 else nc.scalar
                    ea.dma_start(nfb1_sb[0:64, dst], nfull[:, ch:ch + w])
                    nc.gpsimd.dma_start(nfb1_sb[64:128, dst], nfull[:, ch:ch + w])
                # ob needed from step `lag` on; thr only by the final diag units
                nc.scalar.dma_start(ob_sb[:], obf[:, :, :])
                nc.scalar.dma_start(th_sb[:], thr[:, :])

                # sequential mbs sharing one pipeline (mb1 fid overlaps mb0
                # drain); mb0's yt store lands mid-kernel.
                yas = [ytp.tile([128, 512], F32, tag="ya", name=f"ya{mb}")
                       for mb in range(n_mb)]
                orders = []
                for mb in range(n_mb):
                    diag_pairs = (2 * mb, 2 * mb + 1)
                    orders.append(
                        [p for p in range(npair) if p not in diag_pairs]
                        + list(diag_pairs))
                nsteps = n_mb * npair
                # evenly spread DVE-owned slots; mb1 shifted for alternation
                base = {round(i * (npair - 3) / max(n_dve_full - 1, 1))
                        for i in range(n_dve_full)}
                dve_of_mb = [base, {min(p + 1, npair - 3) for p in base}]

                adj_q = {}

                def do_tail(mb):
                    msl = slice(mb * 512, (mb + 1) * 512)
                    ycopy = outp.tile([64, 512], F32, tag="yc")
                    yhalf = outp.tile([64, 512], F32, tag="yh")
                    nc.vector.tensor_copy(ycopy[:], yas[mb][0:64, :])
                    nc.vector.tensor_add(yhalf[:], ycopy[:], yas[mb][64:128, :])
                    nc.sync.dma_start(yt[:, msl], yhalf[:])

                for step in range(nsteps + lag):
                    if step < nsteps:
                        mb, pi = divmod(step, npair)
                        msl = slice(mb * 512, (mb + 1) * 512)
                        pr = orders[mb][pi]
                        kba, kbb = 2 * pr, 2 * pr + 1
                        ca = slice(kba * 128, (kba + 1) * 128)
                        cb = slice(kbb * 128, (kbb + 1) * 128)
                        fps = fidp.tile([128, 1024], F32, tag="fid")
                        nc.tensor.matmul(
                            fps[:, 0:512], nf(0, ca),
                            nl_sb[0:64, msl], start=True, stop=True)
                        nc.tensor.matmul(
                            fps[:, 512:1024], nf(64, cb),
                            nl_sb[64:128, msl], start=True, stop=True)
                        adj = adjp.tile([128, 1024], BF16, tag="adj")
                        if pi >= npair - 2:
                            v = pi - (npair - 2)
                            nc.vector.tensor_tensor(
                                adj[:], fps[:],
                                th_sb[:, v * 1024:(v + 1) * 1024],
                                op=ALU.is_ge)
                        elif pi in dve_of_mb[mb]:
                            nc.vector.tensor_scalar(
                                adj[:], fps[:], SQRT_T, None, op0=ALU.is_ge)
                        else:
                            # one-pass soft threshold: sigmoid(K(d - t))
                            # saturates to exact 0/1 outside +-0.1 of t
                            nc.scalar.activation(
                                adj[:], fps[:], AF.Sigmoid,
                                bias=sbias[:, 0:1], scale=SIG_K)
                        adj_q[step] = adj
                    if step >= lag:
                        q = step - lag
                        mb, qi = divmod(q, npair)
                        pr = orders[mb][qi]
                        adj = adj_q.pop(q)
                        nc.tensor.matmul(
                            yas[mb][0:64, :], ob_sb[:, 2 * pr, :],
                            adj[:, 0:512],
                            start=(qi == 0), stop=(qi == npair - 1),
                            tile_position=(0, 0))
                        nc.tensor.matmul(
                            yas[mb][64:128, :], ob_sb[:, 2 * pr + 1, :],
                            adj[:, 512:1024],
                            start=(qi == 0), stop=(qi == npair - 1),
                            tile_position=(0, 64))
                        if qi == npair - 1:
                            # all-DVE tail; keeps ACT table on Sigmoid
                            do_tail(mb)

    nc.compile()
    return nc


# ---------------------------------------------------------------------------
# Host orchestration
# ---------------------------------------------------------------------------
_CACHE = {}
LAST_RESULTS = {}


def _get(name, builder):
    if name not in _CACHE:
        nc = builder()
        nc.m = get_hw_module(nc.m)
        _CACHE[name] = nc
    return _CACHE[name]


def _phase1_inmaps(x, W1, b1, W2, b2, W3, b3):
    bf = ml_dtypes.bfloat16
    w1 = np.ascontiguousarray(W1.T.reshape(2, 128, 512)).astype(bf)
    w2 = np.ascontiguousarray(W2.T.reshape(4, 128, 256)).astype(bf)
    w3 = np.ascontiguousarray(W3.T.reshape(2, 128, 64)).astype(bf)
    b1h = np.ascontiguousarray(b1.reshape(4, 128).T)
    b2h = np.ascontiguousarray(b2.reshape(2, 128).T)
    b3h = np.ascontiguousarray(b3.reshape(64, 1))
    maps = []
    for c in range(NCORES):
        xT = np.ascontiguousarray(
            x[c * LOCAL:(c + 1) * LOCAL].T.reshape(2, 128, LOCAL)).astype(bf)
        maps.append(dict(x2=xT, w1=w1, w2=w2, w3=w3, b1=b1h, b2=b2h, b3=b3h))
    return maps


def _make_thr():
    thr = np.full((128, 2048), SQRT_T, dtype=np.float32)
    p = np.arange(128)
    thr[p, p] = BIG               # variant 0, A half: diag at col = part
    thr[p, 640 + p] = BIG         # variant 0, B half: col = 512 + 128 + part
    thr[p, 1024 + 256 + p] = BIG  # variant 1, A half
    thr[p, 1024 + 896 + p] = BIG  # variant 1, B half
    return thr.astype(ml_dtypes.bfloat16)


def _phase2_inmaps(normedT_full, out_full):
    f8 = ml_dtypes.float8_e4m3
    out_bf = out_full.astype(ml_dtypes.bfloat16)
    thr = _make_thr()
    n8_full = normedT_full.astype(f8)
    maps = []
    for c in range(NCORES):
        nfull = np.ascontiguousarray(np.roll(n8_full, -LOCAL * c, axis=1))
        ob = np.roll(out_bf, -LOCAL * c, axis=0)
        ob = np.ascontiguousarray(ob.reshape(64, 128, 64).transpose(1, 0, 2))
        nloc = np.ascontiguousarray(n8_full[:, c * LOCAL:(c + 1) * LOCAL])
        maps.append(dict(nfull=nfull, nloc=nloc, obf=ob, thr=thr))
    return maps


def kernel(x, W1, b1, W2, b2, W3, b3, _trace=False):
    x, W1, b1, W2, b2, W3, b3 = [
        np.asarray(a, dtype=np.float32) for a in (x, W1, b1, W2, b2, W3, b3)]
    nc1 = _get("p1", build_phase1)
    nc2 = _get("p2", build_phase2)

    r1 = run_bass_kernel_spmd(nc1, _phase1_inmaps(x, W1, b1, W2, b2, W3, b3),
                              core_ids=list(range(NCORES)), trace=_trace)
    outT_full = np.concatenate([r1.results[c]["outT"] for c in range(NCORES)],
                               axis=1)
    out_full = np.ascontiguousarray(outT_full.T)
    norms = np.linalg.norm(out_full, axis=1, keepdims=True)
    normedT_full = np.ascontiguousarray((out_full / (norms + 1e-12)).T)

    r2 = run_bass_kernel_spmd(nc2, _phase2_inmaps(normedT_full, out_full),
                              core_ids=list(range(NCORES)), trace=_trace)
    y = np.concatenate(
        [np.ascontiguousarray(r2.results[c]["yt"].T) for c in range(NCORES)],
        axis=0)
    LAST_RESULTS["r1"] = r1
    LAST_RESULTS["r2"] = r2
    return y.astype(np.float32)


# revision 5
# speedup vs baseline: 1.0065x; 1.0065x over previous
"""Trainium2 Bass kernel v2 for nn_EstimatorQNN (MLP -> pairwise fidelity graph -> adj @ out).

Contract: kernel(**inputs) takes FULL unsharded inputs (numpy, fp32) and
returns the FULL [8192, 64] fp32 output.  Batch sharded across 8 cores;
host-side all-gather + normalize + fp8 quantize between the two launches.

Phase 1 (per core): outT = MLP(x_local).T          [pure MLP, no norm chain]
Phase 2 (per core): fid block-row = nf_blk.T @ nl (fp8), threshold spread
                    across ACT/DVE/GPS, yT += ob_blk @ adj accumulated.
"""

import numpy as np
import ml_dtypes

import concourse.bass as bass
import concourse.tile as tile
from concourse import bacc, mybir
from concourse.bass_utils import run_bass_kernel_spmd
from concourse.bass_interp import get_hw_module

F32 = mybir.dt.float32
BF16 = mybir.dt.bfloat16
FP8 = mybir.dt.float8e4
AF = mybir.ActivationFunctionType
ALU = mybir.AluOpType

B, D_IN, H1, H2, D_OUT = 8192, 256, 512, 256, 64
NCORES = 8
LOCAL = B // NCORES          # 1024 rows per core
THRESHOLD = 0.9
SQRT_T = float(np.sqrt(np.float32(THRESHOLD)))
BIG = 3.0e4                  # diag-kill threshold (bf16-representable, > any |dot|)


# ---------------------------------------------------------------------------
# Phase 1: MLP only.  Inputs (per core):
#   x2  [2, 128, 1024] bf16 (x_local.T chunked on the 256-dim)
#   w1  [2, 128, 512]  bf16     w2 [4, 128, 256]     w3 [2, 128, 64]
#   b1  [128, 4] f32            b2 [128, 2]          b3 [64, 1]
# Output: outT [64, 1024] f32
# ---------------------------------------------------------------------------
def build_phase1(n_b=LOCAL, reps=1):
    nb = n_b // 512          # batch chunks of 512
    nc = bacc.Bacc("TRN2", target_bir_lowering=False, debug=False,
                   enable_asserts=False, num_devices=NCORES)
    x2 = nc.dram_tensor("x2", [2, 128, n_b], BF16, kind="ExternalInput")
    w1 = nc.dram_tensor("w1", [2, 128, 512], BF16, kind="ExternalInput")
    w2 = nc.dram_tensor("w2", [4, 128, 256], BF16, kind="ExternalInput")
    w3 = nc.dram_tensor("w3", [2, 128, 64], BF16, kind="ExternalInput")
    b1 = nc.dram_tensor("b1", [128, 4], F32, kind="ExternalInput")
    b2 = nc.dram_tensor("b2", [128, 2], F32, kind="ExternalInput")
    b3 = nc.dram_tensor("b3", [64, 1], F32, kind="ExternalInput")
    outT = nc.dram_tensor("outT", [64, n_b], F32, kind="ExternalOutput")

    with tile.TileContext(nc) as tc:
        with (
            tc.tile_pool(name="wpool", bufs=1) as wpool,
            tc.tile_pool(name="hpool", bufs=1) as hpool,
            tc.tile_pool(name="ps128", bufs=4, space="PSUM") as ps128,
            tc.tile_pool(name="ps64", bufs=2, space="PSUM") as ps64,
        ):
            for rep in range(reps):
                xa_sb = wpool.tile([128, 2, 512], BF16, tag="xa")
                xb_sb = wpool.tile([128, 2, n_b - 512], BF16, tag="xb")
                w1_sb = wpool.tile([128, 2, 512], BF16, tag="w1")
                w2_sb = wpool.tile([128, 4, 256], BF16, tag="w2")
                w3_sb = wpool.tile([128, 2, 64], BF16, tag="w3")
                b1_sb = wpool.tile([128, 4], F32, tag="b1")
                b2_sb = wpool.tile([128, 2], F32, tag="b2")
                b3_sb = wpool.tile([64, 1], F32, tag="b3")

                def xs(kc, sl):
                    if sl.stop <= 512:
                        return xa_sb[:, kc, sl]
                    return xb_sb[:, kc, sl.start - 512:sl.stop - 512]

                # critical-path inputs first; first chunk as few DMAs as possible
                nc.sync.dma_start(w1_sb[:, 0, :], w1[0])
                nc.gpsimd.dma_start(xa_sb[:, 0, :], x2[0, :, 0:512])
                nc.sync.dma_start(w1_sb[:, 1, :], w1[1])
                nc.gpsimd.dma_start(xa_sb[:, 1, :], x2[1, :, 0:512])
                nc.scalar.dma_start(b1_sb[:], b1[:, :])
                for kc in range(4):
                    nc.scalar.dma_start(w2_sb[:, kc, :], w2[kc])
                nc.scalar.dma_start(b2_sb[:], b2[:, :])
                for kc in range(2):
                    nc.scalar.dma_start(w3_sb[:, kc, :], w3[kc])
                nc.scalar.dma_start(b3_sb[:], b3[:, :])
                if n_b > 512:
                    nc.gpsimd.dma_start(xb_sb[:, 0, :], x2[0, :, 512:n_b])
                    nc.gpsimd.dma_start(xb_sb[:, 1, :], x2[1, :, 512:n_b])

                h1_sb = hpool.tile([128, 4, n_b], BF16, tag="h1")
                h2_sb = hpool.tile([128, 2, n_b], BF16, tag="h2")
                out_sb = hpool.tile([64, n_b], F32, tag="out")

                # chunk-major pipeline: h2(bb) overlaps h1(bb+1)
                for bb in range(nb):
                    sl = slice(bb * 512, (bb + 1) * 512)
                    # h1T = tanh(W1 @ xT + b1): [512, 512chunk]
                    for hb in range(4):
                        ps = ps128.tile([128, 512], F32, tag="mm")
                        for kc in range(2):
                            nc.tensor.matmul(
                                ps[:],
                                w1_sb[:, kc, hb * 128:(hb + 1) * 128],
                                xs(kc, sl),
                                start=(kc == 0), stop=(kc == 1))
                        nc.scalar.activation(
                            h1_sb[:, hb, sl], ps[:],
                            AF.Tanh, bias=b1_sb[:, hb:hb + 1], scale=1.0)
                    # h2T = tanh(W2 @ h1T + b2): [256, 512chunk]
                    for hb in range(2):
                        ps = ps128.tile([128, 512], F32, tag="mm")
                        for kc in range(4):
                            nc.tensor.matmul(
                                ps[:],
                                w2_sb[:, kc, hb * 128:(hb + 1) * 128],
                                h1_sb[:, kc, sl],
                                start=(kc == 0), stop=(kc == 3))
                        nc.scalar.activation(
                            h2_sb[:, hb, sl], ps[:],
                            AF.Tanh, bias=b2_sb[:, hb:hb + 1], scale=1.0)
                    # outT = W3 @ h2T + b3
                    ps = ps64.tile([64, 512], F32, tag="mmo")
                    for kc in range(2):
                        nc.tensor.matmul(
                            ps[:], w3_sb[:, kc, :], h2_sb[:, kc, sl],
                            start=(kc == 0), stop=(kc == 1))
                    nc.scalar.activation(
                        out_sb[:, sl], ps[:],
                        AF.Identity, bias=b3_sb[:, 0:1], scale=1.0)
                    nc.sync.dma_start(outT[:, sl], out_sb[:, sl])

    nc.compile()
    return nc


# ---------------------------------------------------------------------------
# Phase 2: gram + threshold + adjacency matmul.  Inputs (per core):
#   nfull [64, 8192] fp8  -- normedT_full rolled by -1024*core (k axis)
#   nloc  [64, 1024] fp8  -- this core's normedT slice
#   obf   [128, 64, 64] bf16 -- out_full rolled likewise; [p, kb, d]
#   thr   [128, 2048] bf16 -- 2 diag-kill threshold variants
# Output: yt [64, 1024] f32  (yT for the local rows)
#
# Threshold engine assignment per pair step (npair=32 per mb):
#   diag pairs (2): DVE tensor_tensor is_ge vs thr (one pass from psum)
#   act_set:        ACT Abs -> DVE or GPS bf16 is_ge
#   dve_set:        DVE tensor_scalar is_ge from psum (one-sided)
# ---------------------------------------------------------------------------
SIG_K = 80.0                 # sigmoid sharpness: tail < 1e-8 at margin 0.23


def build_phase2(n_kb=B // 128, n_mb=LOCAL // 512, lag=16, reps=1,
                 n_dve_full=13):
    npair = n_kb // 2
    nc = bacc.Bacc("TRN2", target_bir_lowering=False, debug=False,
                   enable_asserts=False, num_devices=NCORES)
    nfull = nc.dram_tensor("nfull", [64, n_kb * 128], FP8, kind="ExternalInput")
    nloc = nc.dram_tensor("nloc", [64, n_mb * 512], FP8, kind="ExternalInput")
    obf = nc.dram_tensor("obf", [128, n_kb, 64], BF16, kind="ExternalInput")
    thr = nc.dram_tensor("thr", [128, 2048], BF16, kind="ExternalInput")
    yt = nc.dram_tensor("yt", [64, n_mb * 512], F32, kind="ExternalOutput")

    with tile.TileContext(nc) as tc:
        with (
            tc.tile_pool(name="big", bufs=1) as big,
            tc.tile_pool(name="adjp", bufs=lag + 4) as adjp,
            tc.tile_pool(name="absp", bufs=6) as absp,
            tc.tile_pool(name="outp", bufs=2) as outp,
            tc.tile_pool(name="fidp", bufs=3, space="PSUM") as fidp,
            tc.tile_pool(name="ytp", bufs=2, space="PSUM") as ytp,
        ):
            for rep in range(reps):
                # head tile = cols 512:2560 (pairs 2..9 = first 16 steps of
                # the reordered schedule) so the pipeline starts after 1 DMA
                # per half; diag cols 0:512 + the rest live in body tiles.
                HA, HB = 512, 2560
                nfa_sb = big.tile([128, HB - HA], FP8, tag="nfa")
                nfb0_sb = big.tile([128, HA], FP8, tag="nfb0")
                nfb1_sb = big.tile([128, n_kb * 128 - HB], FP8, tag="nfb1")
                nl_sb = big.tile([128, n_mb * 512], FP8, tag="nl")
                ob_sb = big.tile([128, n_kb, 64], BF16, tag="ob")
                th_sb = big.tile([128, 2048], BF16, tag="th")
                sbias = big.tile([128, 1], F32, tag="sbias")
                nc.vector.memset(sbias[:], -SIG_K * SQRT_T)

                def nf(lo, cols):
                    """nfull slice [lo partition half, col range] across tiles."""
                    if HA <= cols.start < HB:
                        return nfa_sb[lo:lo + 64,
                                      cols.start - HA:cols.stop - HA]
                    if cols.start < HA:
                        return nfb0_sb[lo:lo + 64, cols]
                    return nfb1_sb[lo:lo + 64,
                                   cols.start - HB:cols.stop - HB]

                # critical-path first: locals + head tile, few DMAs, then body
                nc.sync.dma_start(nl_sb[0:64, :], nloc[:, :])
                nc.gpsimd.dma_start(nl_sb[64:128, :], nloc[:, :])
                nc.sync.dma_start(nfa_sb[0:64, :], nfull[:, HA:HB])
                nc.gpsimd.dma_start(nfa_sb[64:128, :], nfull[:, HA:HB])
                nc.sync.dma_start(nfb0_sb[0:64, :], nfull[:, 0:HA])
                nc.gpsimd.dma_start(nfb0_sb[64:128, :], nfull[:, 0:HA])
                total = n_kb * 128
                edges = list(range(HB, total, 2048)) + [total]
                for i in range(len(edges) - 1):
                    ch, w = edges[i], edges[i + 1] - edges[i]
                    dst = slice(ch - HB, ch + w - HB)
                    ea = nc.sync if i % 2 == 0 else nc.scalar
                    ea.dma_start(nfb1_sb[0:64, dst], nfull[:, ch:ch + w])
                    nc.gpsimd.dma_start(nfb1_sb[64:128, dst], nfull[:, ch:ch + w])
                # ob needed from step `lag` on; thr only by the final diag units
                nc.scalar.dma_start(ob_sb[:], obf[:, :, :])
                nc.scalar.dma_start(th_sb[:], thr[:, :])

                # sequential mbs sharing one pipeline (mb1 fid overlaps mb0
                # drain); mb0's yt store lands mid-kernel.
                yas = [ytp.tile([128, 512], F32, tag="ya", name=f"ya{mb}")
                       for mb in range(n_mb)]
                orders = []
                for mb in range(n_mb):
                    diag_pairs = (2 * mb, 2 * mb + 1)
                    orders.append(
                        [p for p in range(npair) if p not in diag_pairs]
                        + list(diag_pairs))
                nsteps = n_mb * npair
                # evenly spread DVE-owned slots; mb1 shifted for alternation
                base = {round(i * (npair - 3) / max(n_dve_full - 1, 1))
                        for i in range(n_dve_full)}
                dve_of_mb = [base, {min(p + 1, npair - 3) for p in base}]

                adj_q = {}

                def do_tail(mb):
                    msl = slice(mb * 512, (mb + 1) * 512)
                    ycopy = outp.tile([64, 512], F32, tag="yc")
                    yhalf = outp.tile([64, 512], F32, tag="yh")
                    nc.vector.tensor_copy(ycopy[:], yas[mb][0:64, :])
                    nc.vector.tensor_add(yhalf[:], ycopy[:], yas[mb][64:128, :])
                    nc.sync.dma_start(yt[:, msl], yhalf[:])

                for step in range(nsteps + lag):
                    if step < nsteps:
                        mb, pi = divmod(step, npair)
                        msl = slice(mb * 512, (mb + 1) * 512)
                        pr = orders[mb][pi]
                        kba, kbb = 2 * pr, 2 * pr + 1
                        ca = slice(kba * 128, (kba + 1) * 128)
                        cb = slice(kbb * 128, (kbb + 1) * 128)
                        fps = fidp.tile([128, 1024], F32, tag="fid")
                        nc.tensor.matmul(
                            fps[:, 0:512], nf(0, ca),
                            nl_sb[0:64, msl], start=True, stop=True)
                        nc.tensor.matmul(
                            fps[:, 512:1024], nf(64, cb),
                            nl_sb[64:128, msl], start=True, stop=True)
                        adj = adjp.tile([128, 1024], BF16, tag="adj")
                        if pi >= npair - 2:
                            v = pi - (npair - 2)
                            nc.vector.tensor_tensor(
                                adj[:], fps[:],
                                th_sb[:, v * 1024:(v + 1) * 1024],
                                op=ALU.is_ge)
                        elif pi in dve_of_mb[mb]:
                            nc.vector.tensor_scalar(
                                adj[:], fps[:], SQRT_T, None, op0=ALU.is_ge)
                        else:
                            # one-pass soft threshold: sigmoid(K(d - t))
                            # saturates to exact 0/1 outside +-0.1 of t
                            nc.scalar.activation(
                                adj[:], fps[:], AF.Sigmoid,
                                bias=sbias[:, 0:1], scale=SIG_K)
                        adj_q[step] = adj
                    if step >= lag:
                        q = step - lag
                        mb, qi = divmod(q, npair)
                        pr = orders[mb][qi]
                        adj = adj_q.pop(q)
                        nc.tensor.matmul(
                            yas[mb][0:64, :], ob_sb[:, 2 * pr, :],
                            adj[:, 0:512],
                            start=(qi == 0), stop=(qi == npair - 1),
                            tile_position=(0, 0))
                        nc.tensor.matmul(
                            yas[mb][64:128, :], ob_sb[:, 2 * pr + 1, :],
                            adj[:, 512:1024],
                            start=(qi == 0), stop=(qi == npair - 1),
                            tile_position=(0, 64))
                        if qi == npair - 1:
                            # all-DVE tail; keeps ACT table on Sigmoid
                            do_tail(mb)

    nc.compile()
    return nc


# ---------------------------------------------------------------------------
# Host orchestration
# ---------------------------------------------------------------------------
_CACHE = {}
LAST_RESULTS = {}


def _get(name, builder):
    if name not in _CACHE:
        nc = builder()
        nc.m = get_hw_module(nc.m)
        _CACHE[name] = nc
    return _CACHE[name]


def _phase1_inmaps(x, W1, b1, W2, b2, W3, b3):
    bf = ml_dtypes.bfloat16
    w1 = np.ascontiguousarray(W1.T.reshape(2, 128, 512)).astype(bf)
    w2 = np.ascontiguousarray(W2.T.reshape(4, 128, 256)).astype(bf)
    w3 = np.ascontiguousarray(W3.T.reshape(2, 128, 64)).astype(bf)
    b1h = np.ascontiguousarray(b1.reshape(4, 128).T)
    b2h = np.ascontiguousarray(b2.reshape(2, 128).T)
    b3h = np.ascontiguousarray(b3.reshape(64, 1))
    maps = []
    for c in range(NCORES):
        xT = np.ascontiguousarray(
            x[c * LOCAL:(c + 1) * LOCAL].T.reshape(2, 128, LOCAL)).astype(bf)
        maps.append(dict(x2=xT, w1=w1, w2=w2, w3=w3, b1=b1h, b2=b2h, b3=b3h))
    return maps


def _make_thr():
    thr = np.full((128, 2048), SQRT_T, dtype=np.float32)
    p = np.arange(128)
    thr[p, p] = BIG               # variant 0, A half: diag at col = part
    thr[p, 640 + p] = BIG         # variant 0, B half: col = 512 + 128 + part
    thr[p, 1024 + 256 + p] = BIG  # variant 1, A half
    thr[p, 1024 + 896 + p] = BIG  # variant 1, B half
    return thr.astype(ml_dtypes.bfloat16)


def _phase2_inmaps(normedT_full, out_full):
    f8 = ml_dtypes.float8_e4m3
    out_bf = out_full.astype(ml_dtypes.bfloat16)
    thr = _make_thr()
    n8_full = normedT_full.astype(f8)
    maps = []
    for c in range(NCORES):
        nfull = np.ascontiguousarray(np.roll(n8_full, -LOCAL * c, axis=1))
        ob = np.roll(out_bf, -LOCAL * c, axis=0)
        ob = np.ascontiguousarray(ob.reshape(64, 128, 64).transpose(1, 0, 2))
        nloc = np.ascontiguousarray(n8_full[:, c * LOCAL:(c + 1) * LOCAL])
        maps.append(dict(nfull=nfull, nloc=nloc, obf=ob, thr=thr))
    return maps


def kernel(x, W1, b1, W2, b2, W3, b3, _trace=False):
    x, W1, b1, W2, b2, W3, b3 = [
        np.asarray(a, dtype=np.float32) for a in (x, W1, b1, W2, b2, W3, b3)]
    nc1 = _get("p1", build_phase1)
    nc2 = _get("p2", build_phase2)

    r1 = run_bass_kernel_spmd(nc1, _phase1_inmaps(x, W1, b1, W2, b2, W3, b3),
                              core_ids=list(range(NCORES)), trace=_trace)
    outT_full = np.concatenate([r1.results[c]["outT"] for c in range(NCORES)],
                               axis=1)
    out_full = np.ascontiguousarray(outT_full.T)
    norms = np.linalg.norm(out_full, axis=1, keepdims=True)
    normedT_full = np.ascontiguousarray((out_full / (norms + 1e-12)).T)

    r2 = run_bass_kernel_spmd(nc2, _phase2_inmaps(normedT_full, out_full),
                              core_ids=list(range(NCORES)), trace=_trace)
    y = np.concatenate(
        [np.ascontiguousarray(r2.results[c]["yt"].T) for c in range(NCORES)],
        axis=0)
    LAST_RESULTS["r1"] = r1
    LAST_RESULTS["r2"] = r2
    return y.astype(np.float32)


# revision 7
# speedup vs baseline: 1.0187x; 1.0122x over previous
"""Trainium2 Bass kernel v2 for nn_EstimatorQNN (MLP -> pairwise fidelity graph -> adj @ out).

Contract: kernel(**inputs) takes FULL unsharded inputs (numpy, fp32) and
returns the FULL [8192, 64] fp32 output.  Batch sharded across 8 cores;
host-side all-gather + normalize + fp8 quantize between the two launches.

Phase 1 (per core): outT = MLP(x_local).T          [pure MLP, no norm chain]
Phase 2 (per core): fid block-row = nf_blk.T @ nl (fp8), threshold spread
                    across ACT/DVE/GPS, yT += ob_blk @ adj accumulated.
"""

import numpy as np
import ml_dtypes

import concourse.bass as bass
import concourse.tile as tile
from concourse import bacc, mybir
from concourse.bass_utils import run_bass_kernel_spmd
from concourse.bass_interp import get_hw_module

F32 = mybir.dt.float32
BF16 = mybir.dt.bfloat16
FP8 = mybir.dt.float8e4
AF = mybir.ActivationFunctionType
ALU = mybir.AluOpType

B, D_IN, H1, H2, D_OUT = 8192, 256, 512, 256, 64
NCORES = 8
LOCAL = B // NCORES          # 1024 rows per core
THRESHOLD = 0.9
SQRT_T = float(np.sqrt(np.float32(THRESHOLD)))
BIG = 3.0e4                  # diag-kill threshold (bf16-representable, > any |dot|)


# ---------------------------------------------------------------------------
# Phase 1: MLP only.  Inputs (per core):
#   x2  [2, 128, 1024] bf16 (x_local.T chunked on the 256-dim)
#   w1  [2, 128, 512]  bf16     w2 [4, 128, 256]     w3 [2, 128, 64]
#   b1  [128, 4] f32            b2 [128, 2]          b3 [64, 1]
# Output: outT [64, 1024] f32
# ---------------------------------------------------------------------------
def build_phase1(n_b=LOCAL, reps=1):
    nb = n_b // 512          # batch chunks of 512
    nc = bacc.Bacc("TRN2", target_bir_lowering=False, debug=False,
                   enable_asserts=False, num_devices=NCORES)
    x2 = nc.dram_tensor("x2", [2, 128, n_b], BF16, kind="ExternalInput")
    w1 = nc.dram_tensor("w1", [2, 128, 512], BF16, kind="ExternalInput")
    w2 = nc.dram_tensor("w2", [4, 128, 256], BF16, kind="ExternalInput")
    w3 = nc.dram_tensor("w3", [2, 128, 64], BF16, kind="ExternalInput")
    b1 = nc.dram_tensor("b1", [128, 4], F32, kind="ExternalInput")
    b2 = nc.dram_tensor("b2", [128, 2], F32, kind="ExternalInput")
    b3 = nc.dram_tensor("b3", [64, 1], F32, kind="ExternalInput")
    outT = nc.dram_tensor("outT", [64, n_b], F32, kind="ExternalOutput")

    with tile.TileContext(nc) as tc:
        with (
            tc.tile_pool(name="wpool", bufs=1) as wpool,
            tc.tile_pool(name="hpool", bufs=1) as hpool,
            tc.tile_pool(name="ps128", bufs=4, space="PSUM") as ps128,
            tc.tile_pool(name="ps64", bufs=2, space="PSUM") as ps64,
        ):
            for rep in range(reps):
                xa_sb = wpool.tile([128, 2, 512], BF16, tag="xa")
                xb_sb = wpool.tile([128, 2, n_b - 512], BF16, tag="xb")
                w1_sb = wpool.tile([128, 2, 512], BF16, tag="w1")
                w2_sb = wpool.tile([128, 4, 256], BF16, tag="w2")
                w3_sb = wpool.tile([128, 2, 64], BF16, tag="w3")
                b1_sb = wpool.tile([128, 4], F32, tag="b1")
                b2_sb = wpool.tile([128, 2], F32, tag="b2")
                b3_sb = wpool.tile([64, 1], F32, tag="b3")

                def xs(kc, sl):
                    if sl.stop <= 512:
                        return xa_sb[:, kc, sl]
                    return xb_sb[:, kc, sl.start - 512:sl.stop - 512]

                # critical-path inputs first; first chunk as few DMAs as possible
                nc.sync.dma_start(w1_sb[:, 0, :], w1[0])
                nc.gpsimd.dma_start(xa_sb[:, 0, :], x2[0, :, 0:512])
                nc.sync.dma_start(w1_sb[:, 1, :], w1[1])
                nc.gpsimd.dma_start(xa_sb[:, 1, :], x2[1, :, 0:512])
                nc.scalar.dma_start(b1_sb[:], b1[:, :])
                for kc in range(4):
                    nc.scalar.dma_start(w2_sb[:, kc, :], w2[kc])
                nc.scalar.dma_start(b2_sb[:], b2[:, :])
                for kc in range(2):
                    nc.scalar.dma_start(w3_sb[:, kc, :], w3[kc])
                nc.scalar.dma_start(b3_sb[:], b3[:, :])
                if n_b > 512:
                    nc.gpsimd.dma_start(xb_sb[:, 0, :], x2[0, :, 512:n_b])
                    nc.gpsimd.dma_start(xb_sb[:, 1, :], x2[1, :, 512:n_b])

                h1_sb = hpool.tile([128, 4, n_b], BF16, tag="h1")
                h2_sb = hpool.tile([128, 2, n_b], BF16, tag="h2")
                out_sb = hpool.tile([64, n_b], F32, tag="out")

                # chunk-major pipeline: h2(bb) overlaps h1(bb+1)
                for bb in range(nb):
                    sl = slice(bb * 512, (bb + 1) * 512)
                    # h1T = tanh(W1 @ xT + b1): [512, 512chunk]
                    for hb in range(4):
                        ps = ps128.tile([128, 512], F32, tag="mm")
                        for kc in range(2):
                            nc.tensor.matmul(
                                ps[:],
                                w1_sb[:, kc, hb * 128:(hb + 1) * 128],
                                xs(kc, sl),
                                start=(kc == 0), stop=(kc == 1))
                        nc.scalar.activation(
                            h1_sb[:, hb, sl], ps[:],
                            AF.Tanh, bias=b1_sb[:, hb:hb + 1], scale=1.0)
                    # h2T = tanh(W2 @ h1T + b2): [256, 512chunk]
                    for hb in range(2):
                        ps = ps128.tile([128, 512], F32, tag="mm")
                        for kc in range(4):
                            nc.tensor.matmul(
                                ps[:],
                                w2_sb[:, kc, hb * 128:(hb + 1) * 128],
                                h1_sb[:, kc, sl],
                                start=(kc == 0), stop=(kc == 3))
                        nc.scalar.activation(
                            h2_sb[:, hb, sl], ps[:],
                            AF.Tanh, bias=b2_sb[:, hb:hb + 1], scale=1.0)
                    # outT = W3 @ h2T + b3
                    ps = ps64.tile([64, 512], F32, tag="mmo")
                    for kc in range(2):
                        nc.tensor.matmul(
                            ps[:], w3_sb[:, kc, :], h2_sb[:, kc, sl],
                            start=(kc == 0), stop=(kc == 1))
                    nc.scalar.activation(
                        out_sb[:, sl], ps[:],
                        AF.Identity, bias=b3_sb[:, 0:1], scale=1.0)
                    nc.sync.dma_start(outT[:, sl], out_sb[:, sl])

    nc.compile()
    return nc


# ---------------------------------------------------------------------------
# Phase 2: gram + threshold + adjacency matmul.  Inputs (per core):
#   nfull [64, 8192] fp8  -- normedT_full rolled by -1024*core (k axis)
#   nloc  [64, 1024] fp8  -- this core's normedT slice
#   obf   [128, 64, 64] bf16 -- out_full rolled likewise; [p, kb, d]
#   thr   [128, 2048] bf16 -- 2 diag-kill threshold variants
# Output: yt [64, 1024] f32  (yT for the local rows)
#
# Threshold engine assignment per pair step (npair=32 per mb):
#   diag pairs (2): DVE tensor_tensor is_ge vs thr (one pass from psum)
#   act_set:        ACT Abs -> DVE or GPS bf16 is_ge
#   dve_set:        DVE tensor_scalar is_ge from psum (one-sided)
# ---------------------------------------------------------------------------
SIG_K = 80.0                 # sigmoid sharpness: tail < 1e-8 at margin 0.23


def build_phase2(n_kb=B // 128, n_mb=LOCAL // 512, lag=16, reps=1,
                 n_dve_full=13):
    npair = n_kb // 2
    nc = bacc.Bacc("TRN2", target_bir_lowering=False, debug=False,
                   enable_asserts=False, num_devices=NCORES)
    nfull = nc.dram_tensor("nfull", [64, n_kb * 128], FP8, kind="ExternalInput")
    nloc = nc.dram_tensor("nloc", [64, n_mb * 512], FP8, kind="ExternalInput")
    obf = nc.dram_tensor("obf", [128, n_kb, 64], BF16, kind="ExternalInput")
    thr = nc.dram_tensor("thr", [128, 2048], BF16, kind="ExternalInput")
    yt = nc.dram_tensor("yt", [64, n_mb * 512], F32, kind="ExternalOutput")

    with tile.TileContext(nc) as tc:
        with (
            tc.tile_pool(name="big", bufs=1) as big,
            tc.tile_pool(name="adjp", bufs=lag + 4) as adjp,
            tc.tile_pool(name="absp", bufs=6) as absp,
            tc.tile_pool(name="outp", bufs=2) as outp,
            tc.tile_pool(name="fidp", bufs=3, space="PSUM") as fidp,
            tc.tile_pool(name="ytp", bufs=2, space="PSUM") as ytp,
        ):
            for rep in range(reps):
                # head tile = cols 512:2560 (pairs 2..9 = first 16 steps of
                # the reordered schedule) so the pipeline starts after 1 DMA
                # per half; diag cols 0:512 + the rest live in body tiles.
                HA, HB = 512, 2560
                nfa_sb = big.tile([128, HB - HA], FP8, tag="nfa")
                nfb0_sb = big.tile([128, HA], FP8, tag="nfb0")
                nfb1_sb = big.tile([128, n_kb * 128 - HB], FP8, tag="nfb1")
                nl_sb = big.tile([128, n_mb * 512], FP8, tag="nl")
                ob_sb = big.tile([128, n_kb, 64], BF16, tag="ob")
                th_sb = big.tile([128, 2048], BF16, tag="th")
                sbias = big.tile([128, 1], F32, tag="sbias")
                nc.vector.memset(sbias[:], -SIG_K * SQRT_T)

                def nf(lo, cols):
                    """nfull slice [lo partition half, col range] across tiles."""
                    if HA <= cols.start < HB:
                        return nfa_sb[lo:lo + 64,
                                      cols.start - HA:cols.stop - HA]
                    if cols.start < HA:
                        return nfb0_sb[lo:lo + 64, cols]
                    return nfb1_sb[lo:lo + 64,
                                   cols.start - HB:cols.stop - HB]

                # critical-path first: ONLY locals + head tile are issued
                # before the first pipeline steps are emitted, keeping the
                # batched DMA-completion wait of the first matmuls small.
                nc.sync.dma_start(nl_sb[0:64, :], nloc[:, :])
                nc.gpsimd.dma_start(nl_sb[64:128, :], nloc[:, :])
                nc.sync.dma_start(nfa_sb[0:64, :], nfull[:, HA:HB])
                nc.gpsimd.dma_start(nfa_sb[64:128, :], nfull[:, HA:HB])

                def issue_bulk_dmas():
                    nc.sync.dma_start(nfb0_sb[0:64, :], nfull[:, 0:HA])
                    nc.gpsimd.dma_start(nfb0_sb[64:128, :], nfull[:, 0:HA])
                    total = n_kb * 128
                    edges = list(range(HB, total, 2048)) + [total]
                    for i in range(len(edges) - 1):
                        ch, w = edges[i], edges[i + 1] - edges[i]
                        dst = slice(ch - HB, ch + w - HB)
                        nc.sync.dma_start(nfb1_sb[0:64, dst], nfull[:, ch:ch + w])
                        nc.gpsimd.dma_start(nfb1_sb[64:128, dst],
                                            nfull[:, ch:ch + w])
                    nc.sync.dma_start(ob_sb[:], obf[:, :, :])
                    nc.gpsimd.dma_start(th_sb[:], thr[:, :])

                # sequential mbs sharing one pipeline (mb1 fid overlaps mb0
                # drain); mb0's yt store lands mid-kernel.
                yas = [ytp.tile([128, 512], F32, tag="ya", name=f"ya{mb}")
                       for mb in range(n_mb)]
                orders = []
                for mb in range(n_mb):
                    diag_pairs = (2 * mb, 2 * mb + 1)
                    orders.append(
                        [p for p in range(npair) if p not in diag_pairs]
                        + list(diag_pairs))
                nsteps = n_mb * npair
                # evenly spread DVE-owned slots; mb1 shifted for alternation
                base = {round(i * (npair - 3) / max(n_dve_full - 1, 1))
                        for i in range(n_dve_full)}
                dve_of_mb = [base, {min(p + 1, npair - 3) for p in base}]

                adj_q = {}

                def do_tail(mb):
                    msl = slice(mb * 512, (mb + 1) * 512)
                    ycopy = outp.tile([64, 512], F32, tag="yc")
                    yhalf = outp.tile([64, 512], F32, tag="yh")
                    nc.vector.tensor_copy(ycopy[:], yas[mb][0:64, :])
                    nc.vector.tensor_add(yhalf[:], ycopy[:], yas[mb][64:128, :])
                    nc.sync.dma_start(yt[:, msl], yhalf[:])

                for step in range(nsteps + lag):
                    if step < nsteps:
                        mb, pi = divmod(step, npair)
                        msl = slice(mb * 512, (mb + 1) * 512)
                        pr = orders[mb][pi]
                        kba, kbb = 2 * pr, 2 * pr + 1
                        ca = slice(kba * 128, (kba + 1) * 128)
                        cb = slice(kbb * 128, (kbb + 1) * 128)
                        fps = fidp.tile([128, 1024], F32, tag="fid")
                        nc.tensor.matmul(
                            fps[:, 0:512], nf(0, ca),
                            nl_sb[0:64, msl], start=True, stop=True)
                        nc.tensor.matmul(
                            fps[:, 512:1024], nf(64, cb),
                            nl_sb[64:128, msl], start=True, stop=True)
                        adj = adjp.tile([128, 1024], BF16, tag="adj")
                        if pi >= npair - 2:
                            v = pi - (npair - 2)
                            nc.vector.tensor_tensor(
                                adj[:], fps[:],
                                th_sb[:, v * 1024:(v + 1) * 1024],
                                op=ALU.is_ge)
                        elif pi in dve_of_mb[mb]:
                            nc.vector.tensor_scalar(
                                adj[:], fps[:], SQRT_T, None, op0=ALU.is_ge)
                        else:
                            # one-pass soft threshold: sigmoid(K(d - t))
                            # saturates to exact 0/1 outside +-0.1 of t
                            nc.scalar.activation(
                                adj[:], fps[:], AF.Sigmoid,
                                bias=sbias[:, 0:1], scale=SIG_K)
                        adj_q[step] = adj
                        if step == 3:
                            # bulk loads issued after the pipeline is rolling
                            issue_bulk_dmas()
                    if step >= lag:
                        q = step - lag
                        mb, qi = divmod(q, npair)
                        pr = orders[mb][qi]
                        adj = adj_q.pop(q)
                        nc.tensor.matmul(
                            yas[mb][0:64, :], ob_sb[:, 2 * pr, :],
                            adj[:, 0:512],
                            start=(qi == 0), stop=(qi == npair - 1),
                            tile_position=(0, 0))
                        nc.tensor.matmul(
                            yas[mb][64:128, :], ob_sb[:, 2 * pr + 1, :],
                            adj[:, 512:1024],
                            start=(qi == 0), stop=(qi == npair - 1),
                            tile_position=(0, 64))
                        if qi == npair - 1:
                            # all-DVE tail; keeps ACT table on Sigmoid
                            do_tail(mb)

    nc.compile()
    return nc


# ---------------------------------------------------------------------------
# Host orchestration
# ---------------------------------------------------------------------------
_CACHE = {}
LAST_RESULTS = {}


def _get(name, builder):
    if name not in _CACHE:
        nc = builder()
        nc.m = get_hw_module(nc.m)
        _CACHE[name] = nc
    return _CACHE[name]


def _phase1_inmaps(x, W1, b1, W2, b2, W3, b3):
    bf = ml_dtypes.bfloat16
    w1 = np.ascontiguousarray(W1.T.reshape(2, 128, 512)).astype(bf)
    w2 = np.ascontiguousarray(W2.T.reshape(4, 128, 256)).astype(bf)
    w3 = np.ascontiguousarray(W3.T.reshape(2, 128, 64)).astype(bf)
    b1h = np.ascontiguousarray(b1.reshape(4, 128).T)
    b2h = np.ascontiguousarray(b2.reshape(2, 128).T)
    b3h = np.ascontiguousarray(b3.reshape(64, 1))
    maps = []
    for c in range(NCORES):
        xT = np.ascontiguousarray(
            x[c * LOCAL:(c + 1) * LOCAL].T.reshape(2, 128, LOCAL)).astype(bf)
        maps.append(dict(x2=xT, w1=w1, w2=w2, w3=w3, b1=b1h, b2=b2h, b3=b3h))
    return maps


def _make_thr():
    thr = np.full((128, 2048), SQRT_T, dtype=np.float32)
    p = np.arange(128)
    thr[p, p] = BIG               # variant 0, A half: diag at col = part
    thr[p, 640 + p] = BIG         # variant 0, B half: col = 512 + 128 + part
    thr[p, 1024 + 256 + p] = BIG  # variant 1, A half
    thr[p, 1024 + 896 + p] = BIG  # variant 1, B half
    return thr.astype(ml_dtypes.bfloat16)


def _phase2_inmaps(normedT_full, out_full):
    f8 = ml_dtypes.float8_e4m3
    out_bf = out_full.astype(ml_dtypes.bfloat16)
    thr = _make_thr()
    n8_full = normedT_full.astype(f8)
    maps = []
    for c in range(NCORES):
        nfull = np.ascontiguousarray(np.roll(n8_full, -LOCAL * c, axis=1))
        ob = np.roll(out_bf, -LOCAL * c, axis=0)
        ob = np.ascontiguousarray(ob.reshape(64, 128, 64).transpose(1, 0, 2))
        nloc = np.ascontiguousarray(n8_full[:, c * LOCAL:(c + 1) * LOCAL])
        maps.append(dict(nfull=nfull, nloc=nloc, obf=ob, thr=thr))
    return maps


def kernel(x, W1, b1, W2, b2, W3, b3, _trace=False):
    x, W1, b1, W2, b2, W3, b3 = [
        np.asarray(a, dtype=np.float32) for a in (x, W1, b1, W2, b2, W3, b3)]
    nc1 = _get("p1", build_phase1)
    nc2 = _get("p2", build_phase2)

    r1 = run_bass_kernel_spmd(nc1, _phase1_inmaps(x, W1, b1, W2, b2, W3, b3),
                              core_ids=list(range(NCORES)), trace=_trace)
    outT_full = np.concatenate([r1.results[c]["outT"] for c in range(NCORES)],
                               axis=1)
    out_full = np.ascontiguousarray(outT_full.T)
    norms = np.linalg.norm(out_full, axis=1, keepdims=True)
    normedT_full = np.ascontiguousarray((out_full / (norms + 1e-12)).T)

    r2 = run_bass_kernel_spmd(nc2, _phase2_inmaps(normedT_full, out_full),
                              core_ids=list(range(NCORES)), trace=_trace)
    y = np.concatenate(
        [np.ascontiguousarray(r2.results[c]["yt"].T) for c in range(NCORES)],
        axis=0)
    LAST_RESULTS["r1"] = r1
    LAST_RESULTS["r2"] = r2
    return y.astype(np.float32)
